# revision 9
# baseline (speedup 1.0000x reference)
"""Trainium2 Bass kernel for a dense transformer block (B=2, T=2048, C=1024, H=16).

Sharding: 8-way tensor parallel over heads for attention (each core computes
QKV + causal attention for 2 heads over all 4096 tokens), then a single
8-rank AllToAll switches to token parallelism (512 tokens per core) for the
attention projection, residual, LN2 and the MLP.

Layout: activations are kept transposed on-device ([feature, token], feature
on partitions) so every matmul is a natural lhsT.T @ rhs with no on-device
transposes (except V, which needs [token, dim] blocks for the AV matmul).
LayerNorm gains/biases are folded into the following weight matrices on the
host; the normalization itself is applied as a post-matmul fixup:
    W'.T @ ((x - mu) * rstd) = (W'.T @ x) * rstd - colsum(W') * (mu * rstd)
which is exact and avoids materializing the normalized activations for QKV.

Matmuls run in float32r (11-bit mantissa, full PE rate at 512-wide moving
operands); every PE operand is produced in f32r (host pre-rounds DRAM-fed
tensors with the exact fp32->fp32r rounding).

DMA dispatch is the scarce resource (~625 ns serialized per dispatch per
HWDGE queue): weights are laid out host-side so each consumer group is one
large contiguous DMA, and collective-dependent loads go on the Activation
HWDGE queue so the SP queue is never head-of-line blocked by the AllToAll.
"""

import math
import sys
from contextlib import ExitStack

import numpy as np

for _p in ("/opt/trn_rl_repo",):
    if _p not in sys.path:
        sys.path.insert(0, _p)

import concourse.bacc as bacc
import concourse.mybir as mybir
import concourse.tile as tile
from concourse.bass_utils import run_bass_kernel_spmd

F32 = mybir.dt.float32
F32R = mybir.dt.float32r

B, T, C = 2, 2048, 1024
H, HD = 16, 64
TT = B * T              # 4096 flat tokens
NCORES = 8
CHUNK = TT // NCORES    # 512 tokens per core for the MLP part
NC_BLK = C // 128       # 8 feature blocks
NF_BLK = 4 * C // 128   # 32 mlp-hidden blocks
EPS = 1e-5


def build_program(repeat=1, phases=99):
    nc = bacc.Bacc("TRN2", target_bir_lowering=False, debug=False,
                   num_devices=NCORES)

    # ---- I/O ----
    xT = nc.dram_tensor("xT", [C, TT], F32R, kind="ExternalInput")
    xc = nc.dram_tensor("xc", [C, CHUNK], F32, kind="ExternalInput")
    wqkv = nc.dram_tensor("wqkv", [NC_BLK, 3, 128, 128], F32R, kind="ExternalInput")
    bqkv = nc.dram_tensor("bqkv", [128, 3], F32, kind="ExternalInput")
    cqkv = nc.dram_tensor("cqkv", [128, 3], F32, kind="ExternalInput")
    # weight blocks are laid out so each consumer group is contiguous:
    # wproj[co][ci], wfc[f][c], wmlp[co][f]
    wproj = nc.dram_tensor("wproj", [NC_BLK, NC_BLK, 128, 128], F32R, kind="ExternalInput")
    bproj = nc.dram_tensor("bproj", [128, NC_BLK], F32, kind="ExternalInput")
    wfc = nc.dram_tensor("wfc", [NF_BLK, NC_BLK, 128, 128], F32R, kind="ExternalInput")
    bfc = nc.dram_tensor("bfc", [128, NF_BLK], F32, kind="ExternalInput")
    wmlp = nc.dram_tensor("wmlp", [NC_BLK, NF_BLK, 128, 128], F32R, kind="ExternalInput")
    bmlp = nc.dram_tensor("bmlp", [128, NC_BLK], F32, kind="ExternalInput")
    ones1 = nc.dram_tensor("ones1", [128, 128], F32R, kind="ExternalInput")
    cmask = nc.dram_tensor("cmask", [128, 4, 512], F32R, kind="ExternalInput")
    identin = nc.dram_tensor("identin", [128, 128], F32, kind="ExternalInput")
    epsin = nc.dram_tensor("epsin", [128, 1], F32, kind="ExternalInput")
    yout = nc.dram_tensor("yout", [C, CHUNK], F32, kind="ExternalOutput")

    AFT = mybir.ActivationFunctionType
    ALU = mybir.AluOpType

    with tile.TileContext(nc) as tc, ExitStack() as top:
        psum = top.enter_context(tc.tile_pool(name="psum", bufs=1, space="PSUM"))
        consts = top.enter_context(tc.tile_pool(name="consts", bufs=1))
        rows_pool = top.enter_context(tc.tile_pool(name="rows", bufs=6))
        bcast_pool = top.enter_context(tc.tile_pool(name="bcast", bufs=4))
        work = top.enter_context(tc.tile_pool(name="work", bufs=6))
        dram = top.enter_context(tc.tile_pool(name="dram", bufs=1, space="DRAM"))

        # ---- constants (all DMA'd from host) ----
        ident = consts.tile([128, 128], F32)
        nc.sync.dma_start(out=ident, in_=identin[:, :])
        ones_row = consts.tile([128, 128], F32R)
        nc.sync.dma_start(out=ones_row, in_=ones1[:, :])
        ones_col = ones_row[:, 0:1]
        eps_col = consts.tile([128, 1], F32)
        nc.sync.dma_start(out=eps_col, in_=epsin[:, :])
        masks = consts.tile([128, 4, 512], F32R)
        nc.sync.dma_start(out=masks, in_=cmask[:, :, :])
        sb_bqkv = consts.tile([128, 3], F32)
        nc.sync.dma_start(out=sb_bqkv, in_=bqkv[:, :])
        sb_cqkv = consts.tile([128, 3], F32)
        nc.sync.dma_start(out=sb_cqkv, in_=cqkv[:, :])
        sb_bproj = consts.tile([128, NC_BLK], F32)
        nc.sync.dma_start(out=sb_bproj, in_=bproj[:, :])
        sb_bfc = consts.tile([128, NF_BLK], F32)
        nc.sync.dma_start(out=sb_bfc, in_=bfc[:, :])
        sb_bmlp = consts.tile([128, NC_BLK], F32)
        nc.sync.dma_start(out=sb_bmlp, in_=bmlp[:, :])
        # QKV weight slice is small (1.5 MB) - resident for the whole kernel
        wq_all = consts.tile([128, NC_BLK, 3, 128], F32R)
        nc.sync.dma_start(out=wq_all,
                          in_=wqkv.ap().rearrange("c j p k -> p c j k"))

        def bcast_row(row_ap, nparts=128):
            """Broadcast a [1, 512] SBUF row to [nparts, 512] via a K=1 PE
            outer product with a ones row, evacuated to SBUF by DVE."""
            ps = psum.tile([nparts, 512], F32, tag="mm", bufs=4, name="ps_bc")
            nc.tensor.matmul(ps, ones_row[0:1, 0:nparts], row_ap,
                             start=True, stop=True)
            out = bcast_pool.tile([nparts, 512], F32, tag="bc", name="bc_row")
            nc.vector.tensor_copy(out, ps)
            return out

        def ln_stats_rows(src_slices):
            """src_slices: NC_BLK [128, 512] f32r APs (feature blocks of one
            512-token chunk). Returns (rstd_b, nmr_b) [128, 512] broadcast
            tiles holding rstd and -mu*rstd per token."""
            ps_s = psum.tile([1, 512], F32, tag="rows_ps", bufs=2)
            ps_q = psum.tile([1, 512], F32, tag="rows_ps", bufs=2)
            for c in range(NC_BLK):
                nc.tensor.matmul(ps_s, ones_col, src_slices[c],
                                 start=(c == 0), stop=(c == NC_BLK - 1))
            for c in range(NC_BLK):
                sq = work.tile([128, 512], F32R, tag="wk")
                nc.scalar.activation(out=sq, in_=src_slices[c],
                                     func=AFT.Square)
                nc.tensor.matmul(ps_q, ones_col, sq,
                                 start=(c == 0), stop=(c == NC_BLK - 1))
            mu = rows_pool.tile([1, 512], F32, tag="r")
            nc.vector.tensor_scalar_mul(mu, ps_s[0:1, :], 1.0 / C)
            ex2 = rows_pool.tile([1, 512], F32, tag="r")
            nc.vector.tensor_scalar_mul(ex2, ps_q[0:1, :], 1.0 / C)
            # var = ex2 - mu*mu
            var = rows_pool.tile([1, 512], F32, tag="r")
            musq = rows_pool.tile([1, 512], F32, tag="r")
            nc.vector.tensor_mul(musq, mu, mu)
            nc.vector.tensor_sub(var, ex2, musq)
            sd = rows_pool.tile([1, 512], F32, tag="r")
            nc.scalar.activation(out=sd, in_=var, func=AFT.Sqrt,
                                 bias=eps_col[0:1, 0:1])
            rstd = rows_pool.tile([1, 512], F32R, tag="r")
            with nc.allow_low_precision(reason="f32r is full-width"):
                nc.vector.reciprocal(rstd, sd)
            nmr = rows_pool.tile([1, 512], F32R, tag="r")
            nc.vector.tensor_mul(nmr, mu, rstd)
            nc.vector.tensor_scalar_mul(nmr, nmr, -1.0)
            return bcast_row(rstd[0:1, :]), bcast_row(nmr[0:1, :])

        def emit_body(rep):
            with ExitStack() as attn_scope:
                attn_pool = attn_scope.enter_context(
                    tc.tile_pool(name=f"attn{rep}", bufs=1))
                # qkvT[:, j, t]: j=0 Q, 1 K, 2 V; rows 0:64 head A, 64:128 head B
                qkvT = attn_pool.tile([128, 3, TT], F32R, name="qkvT")
                # V in natural [token, dim] blocks plus an appended ones col
                vones = attn_pool.tile([128, 2, TT // 128, 65], F32R,
                                       name="vones")
                yT = attn_pool.tile([128, 2, TT], F32R, name="yT")

                # ========== Phase 1: LN1 stats + QKV ==========
                with ExitStack() as p1_scope:
                    xc_pool = p1_scope.enter_context(
                        tc.tile_pool(name=f"xcp{rep}", bufs=2))
                    for qi in range(8):
                        xTc = xc_pool.tile([128, NC_BLK, 512], F32R, tag="xTc",
                                           name="xTc")
                        nc.sync.dma_start(
                            out=xTc,
                            in_=xT[:, 512 * qi:512 * (qi + 1)].rearrange(
                                "(c p) t -> p c t", p=128))
                        srcs = [xTc[:, c, :] for c in range(NC_BLK)]
                        rstd_b, nmr_b = ln_stats_rows(srcs)
                        for j in range(3):
                            ps = psum.tile([128, 512], F32, tag="mm", bufs=4,
                                           name="ps_qkv")
                            for c in range(NC_BLK):
                                nc.tensor.matmul(ps, wq_all[:, c, j, :],
                                                 srcs[c],
                                                 start=(c == 0),
                                                 stop=(c == NC_BLK - 1))
                            t1 = work.tile([128, 512], F32, tag="wk", name="t1")
                            nc.vector.tensor_mul(t1, ps, rstd_b)
                            t2 = work.tile([128, 512], F32, tag="wk", name="t2")
                            nc.vector.scalar_tensor_tensor(
                                out=t2, in0=nmr_b, scalar=sb_cqkv[:, j:j + 1],
                                in1=t1, op0=ALU.mult, op1=ALU.add)
                            nc.scalar.activation(
                                out=qkvT[:, j, 512 * qi:512 * (qi + 1)],
                                in_=t2, func=AFT.Identity,
                                bias=sb_bqkv[:, j:j + 1])
                if phases <= 1:
                    return

                # V -> natural [token, dim] blocks (PE transpose) + ones col
                nc.sync.dma_start(out=vones[:, :, :, 64:65],
                                  in_=ones1[:, 0:64])
                for kb in range(TT // 128):
                    ps_t = psum.tile([128, 128], F32, tag="mm", bufs=4,
                                     name="ps_t")
                    nc.tensor.transpose(
                        ps_t, qkvT[:, 2, 128 * kb:128 * (kb + 1)].bitcast(F32),
                        ident)
                    for hh in range(2):
                        nc.vector.tensor_copy(vones[:, hh, kb, 0:64],
                                              ps_t[:, 64 * hh:64 * hh + 64])
                if phases <= 2:
                    return

                # ========== Phase 2: causal attention (head-major) ==========
                a2a_in = [dram.tile([NCORES, 64, CHUNK], F32,
                                    name=f"a2a_in{hh}") for hh in range(2)]
                a2a_out = [dram.tile([NCORES, 64, CHUNK], F32,
                                     name=f"a2a_out{hh}") for hh in range(2)]
                inv_sqrt_hd = 1.0 / math.sqrt(HD)
                for hh in range(2):
                    hsl = slice(64 * hh, 64 * hh + 64)
                    for b in range(B):
                        for ql in range(4):
                            nk = 4 * ql + 4
                            q_sl = slice(T * b + 512 * ql,
                                         T * b + 512 * (ql + 1))
                            ps_y = psum.tile([65, 512], F32, tag="av", bufs=2,
                                             name="ps_y")
                            for k in range(nk):
                                k_sl = slice(T * b + 128 * k,
                                             T * b + 128 * (k + 1))
                                ps_st = psum.tile([128, 512], F32, tag="mm",
                                                  bufs=4, name="ps_st")
                                nc.tensor.matmul(ps_st, qkvT[hsl, 1, k_sl],
                                                 qkvT[hsl, 0, q_sl],
                                                 start=True, stop=True)
                                est = work.tile([128, 512], F32R, tag="est",
                                                name="est")
                                nc.scalar.activation(out=est, in_=ps_st,
                                                     func=AFT.Exp,
                                                     scale=inv_sqrt_hd)
                                m = k - 4 * ql
                                if m >= 0:
                                    nc.vector.tensor_mul(est, est,
                                                         masks[:, m, :])
                                nc.tensor.matmul(
                                    ps_y[0:65, :],
                                    vones[:, hh, (T * b) // 128 + k, :], est,
                                    start=(k == 0), stop=(k == nk - 1))
                            # normalize: recip of the sum row (partition 64),
                            # broadcast down via K=1 matmul at row group 64
                            srow = work.tile([128, 512], F32R, tag="wk",
                                             name="srow")
                            with nc.allow_low_precision(reason="f32r"):
                                nc.vector.reciprocal(srow[64:65, :],
                                                     ps_y[64:65, :])
                            ps_rb = psum.tile([64, 512], F32, tag="mm", bufs=4,
                                              name="ps_rb")
                            nc.tensor.matmul(ps_rb[0:64, :],
                                             ones_row[64:65, 0:64],
                                             srow[64:65, :],
                                             start=True, stop=True)
                            rb = bcast_pool.tile([64, 512], F32, tag="bc",
                                                 name="rb")
                            nc.vector.tensor_copy(rb[0:64, :], ps_rb[0:64, :])
                            nc.vector.tensor_mul(yT[0:64, hh, q_sl],
                                                 ps_y[0:64, :], rb[0:64, :])
                    if phases <= 3:
                        continue
                    # ======== Phase 3: AllToAll for this head ========
                    nc.sync.dma_start(
                        out=a2a_in[hh].rearrange("j p t -> p j t"),
                        in_=yT[0:64, hh, :].bitcast(F32).rearrange(
                            "p (j t) -> p j t", t=CHUNK))
                    nc.gpsimd.collective_compute(
                        "AllToAll", ALU.bypass,
                        replica_groups=[list(range(NCORES))],
                        ins=[a2a_in[hh][:]], outs=[a2a_out[hh][:]],
                    )

            if phases <= 4:
                return
            # attention pool released; MLP-phase persistents reuse the space
            with ExitStack() as mlp_scope:
                mlp_pool = mlp_scope.enter_context(
                    tc.tile_pool(name=f"mlp{rep}", bufs=1))
                residT = mlp_pool.tile([128, NC_BLK, CHUNK], F32R,
                                       name="residT")
                ln2T = mlp_pool.tile([128, NC_BLK, CHUNK], F32R, name="ln2T")
                hT = mlp_pool.tile([128, NF_BLK, CHUNK], F32R, name="hT")

                with ExitStack() as p4_scope:
                    p4_pool = p4_scope.enter_context(
                        tc.tile_pool(name=f"p4_{rep}", bufs=1))
                    # prefetches that must NOT wait for the collective go on
                    # the SP queue right away
                    xct = p4_pool.tile([128, NC_BLK, CHUNK], F32, name="xct")
                    nc.sync.dma_start(
                        out=xct, in_=xc.ap().rearrange("(c p) t -> p c t",
                                                       p=128))
                    wp = []
                    for co in range(NC_BLK):
                        w = p4_pool.tile([128, NC_BLK, 128], F32R, tag="wpj",
                                         bufs=2, name=f"wp{co}")
                        nc.sync.dma_start(
                            out=w, in_=wproj[co].rearrange("ci p k -> p ci k"))
                        wp.append(w)
                    # collective-dependent load on the ACT HWDGE queue so the
                    # SP queue isn't head-of-line blocked
                    yfull = p4_pool.tile([128, NC_BLK, CHUNK], F32R,
                                         name="yfull")
                    for hh in range(2):
                        nc.scalar.dma_start(
                            out=yfull[64 * hh:64 * hh + 64, :, :],
                            in_=a2a_out[hh].bitcast(F32R).rearrange(
                                "j p t -> p j t"))

                    # ===== Phase 4: attn proj + residual =====
                    for co in range(NC_BLK):
                        ps = psum.tile([128, 512], F32, tag="mm", bufs=4,
                                       name="ps_pj")
                        for ci in range(NC_BLK):
                            nc.tensor.matmul(ps, wp[co][:, ci, :],
                                             yfull[:, ci, :],
                                             start=(ci == 0),
                                             stop=(ci == NC_BLK - 1))
                        nc.vector.scalar_tensor_tensor(
                            out=residT[:, co, :], in0=ps,
                            scalar=sb_bproj[:, co:co + 1],
                            in1=xct[:, co, :], op0=ALU.add, op1=ALU.add)
                    if phases <= 5:
                        return

                    # ===== Phase 5: LN2 =====
                    rstd2_b, nmr2_b = ln_stats_rows(
                        [residT[:, c, :] for c in range(NC_BLK)])
                    for c in range(NC_BLK):
                        t = work.tile([128, 512], F32, tag="wk", name="t5")
                        nc.vector.tensor_mul(t, residT[:, c, :], rstd2_b)
                        nc.vector.tensor_add(ln2T[:, c, :], t, nmr2_b)
                if phases <= 6:
                    return

                # ===== Phase 6/7: MLP =====
                with ExitStack() as p6_scope:
                    p6_pool = p6_scope.enter_context(
                        tc.tile_pool(name=f"p6_{rep}", bufs=1))
                    for f in range(NF_BLK):
                        wf = p6_pool.tile([128, NC_BLK, 128], F32R, tag="wfc",
                                          bufs=3, name="wf")
                        nc.sync.dma_start(
                            out=wf, in_=wfc[f].rearrange("c p k -> p c k"))
                        ps = psum.tile([128, 512], F32, tag="mm", bufs=4,
                                       name="ps_fc")
                        for c in range(NC_BLK):
                            nc.tensor.matmul(ps, wf[:, c, :], ln2T[:, c, :],
                                             start=(c == 0),
                                             stop=(c == NC_BLK - 1))
                        nc.scalar.activation(out=hT[:, f, :], in_=ps,
                                             func=AFT.Gelu,
                                             bias=sb_bfc[:, f:f + 1])
                    if phases <= 7:
                        return

                    for co in range(NC_BLK):
                        wm = p6_pool.tile([128, NF_BLK, 128], F32R, tag="wm",
                                          bufs=2, name="wm")
                        nc.sync.dma_start(
                            out=wm, in_=wmlp[co].rearrange("f p k -> p f k"))
                        ps = psum.tile([128, 512], F32, tag="mm", bufs=4,
                                       name="ps_mp")
                        for f in range(NF_BLK):
                            nc.tensor.matmul(ps, wm[:, f, :], hT[:, f, :],
                                             start=(f == 0),
                                             stop=(f == NF_BLK - 1))
                        yo = work.tile([128, 512], F32, tag="wk", name="yo")
                        nc.vector.scalar_tensor_tensor(
                            out=yo, in0=ps, scalar=sb_bmlp[:, co:co + 1],
                            in1=residT[:, co, :], op0=ALU.add, op1=ALU.add)
                        nc.sync.dma_start(
                            out=yout[128 * co:128 * (co + 1), :], in_=yo)

        for _rep in range(repeat):
            emit_body(_rep)

    nc.compile()
    return nc


_NC_CACHE = {}


def _get_program(repeat=1):
    if repeat not in _NC_CACHE:
        _NC_CACHE[repeat] = build_program(repeat)
    return _NC_CACHE[repeat]


def round_f32r(a):
    """Round an f32 array to the PE's fp32r format (11-bit mantissa,
    round-to-nearest), matching walrus's fp32_to_fp32r."""
    bits = np.ascontiguousarray(a, np.float32).view(np.uint32)
    return ((bits + np.uint32(0x800)) & np.uint32(0xFFFFF000)).view(np.float32)


def prepare_inputs(x, ln1_g, ln1_b, w_attn, b_attn, w_attn_proj, b_attn_proj,
                   ln2_g, ln2_b, w_fc, b_fc, w_mlp_proj, b_mlp_proj):
    """Host-side fold/slice/block. Returns in_maps (list of 8 dicts)."""
    f = np.float32
    x = np.asarray(x, f)
    # fold LN1 into w_attn, LN2 into w_fc
    w_attn_e = (np.asarray(ln1_g, f)[:, None] * np.asarray(w_attn, f))
    b_attn_e = np.asarray(ln1_b, f) @ np.asarray(w_attn, f) + np.asarray(b_attn, f)
    w_fc_e = (np.asarray(ln2_g, f)[:, None] * np.asarray(w_fc, f))
    b_fc_e = np.asarray(ln2_b, f) @ np.asarray(w_fc, f) + np.asarray(b_fc, f)
    colsum = w_attn_e.sum(axis=0, dtype=np.float64).astype(f)

    xT = np.ascontiguousarray(
        np.concatenate([x[0].T, x[1].T], axis=1), dtype=f)  # [C, 4096]
    xT_r = round_f32r(xT)  # PE-facing copy, pre-rounded to fp32r

    def blk(w, nr, ncb):
        # [nr*128, ncb*128] -> [nr, ncb, 128, 128]
        return w.reshape(nr, 128, ncb, 128).transpose(0, 2, 1, 3)

    # consumer-contiguous layouts: wproj[co][ci], wfc[f][c], wmlp[co][f]
    wproj_r = round_f32r(np.ascontiguousarray(
        blk(np.asarray(w_attn_proj, f), NC_BLK, NC_BLK).transpose(1, 0, 2, 3)))
    wfc_r = round_f32r(np.ascontiguousarray(
        blk(w_fc_e, NC_BLK, NF_BLK).transpose(1, 0, 2, 3)))
    wmlp_r = round_f32r(np.ascontiguousarray(
        blk(np.asarray(w_mlp_proj, f), NF_BLK, NC_BLK).transpose(1, 0, 2, 3)))

    def rows_t(v, nb):
        return np.ascontiguousarray(np.asarray(v, f).reshape(nb, 128).T)

    bproj_t = rows_t(b_attn_proj, NC_BLK)
    bfc_t = rows_t(b_fc_e, NF_BLK)
    bmlp_t = rows_t(b_mlp_proj, NC_BLK)

    ones_arr = np.ones((128, 128), f)
    ident_arr = np.eye(128, dtype=f)
    eps_arr = np.full((128, 1), EPS, f)
    # causal mask m: [p, c] valid (1.0) iff p + 128*m <= c
    p_idx = np.arange(128)[:, None]
    c_idx = np.arange(512)[None, :]
    cmask_arr = np.ascontiguousarray(np.stack(
        [(p_idx + 128 * m <= c_idx).astype(f) for m in range(4)], axis=1))

    in_maps = []
    for i in range(NCORES):
        qcols = slice(128 * i, 128 * (i + 1))
        kcols = slice(C + 128 * i, C + 128 * (i + 1))
        vcols = slice(2 * C + 128 * i, 2 * C + 128 * (i + 1))
        wq = np.empty((NC_BLK, 3, 128, 128), f)
        for c in range(NC_BLK):
            rsl = slice(128 * c, 128 * (c + 1))
            wq[c, 0] = w_attn_e[rsl, qcols]
            wq[c, 1] = w_attn_e[rsl, kcols]
            wq[c, 2] = w_attn_e[rsl, vcols]
        bq = np.stack([b_attn_e[qcols], b_attn_e[kcols], b_attn_e[vcols]], axis=1)
        cq = np.stack([colsum[qcols], colsum[kcols], colsum[vcols]], axis=1)
        in_maps.append({
            "ones1": ones_arr,
            "cmask": cmask_arr,
            "identin": ident_arr,
            "epsin": eps_arr,
            "xT": xT_r,
            "xc": np.ascontiguousarray(xT[:, CHUNK * i:CHUNK * (i + 1)]),
            "wqkv": round_f32r(wq),
            "bqkv": np.ascontiguousarray(bq),
            "cqkv": np.ascontiguousarray(cq),
            "wproj": wproj_r,
            "bproj": bproj_t,
            "wfc": wfc_r,
            "bfc": bfc_t,
            "wmlp": wmlp_r,
            "bmlp": bmlp_t,
        })
    return in_maps


def assemble_output(results):
    out = np.empty((B, T, C), np.float32)
    for i in range(NCORES):
        bidx = i // 4
        tsl = slice(512 * (i % 4), 512 * (i % 4 + 1))
        out[bidx, tsl, :] = results[i]["yout"].T
    return out


def kernel(**inputs):
    nc = _get_program()
    in_maps = prepare_inputs(**inputs)
    res = run_bass_kernel_spmd(nc, in_maps, list(range(NCORES)))
    return assemble_output(res.results)


if __name__ == "__main__":
    import reference
    inputs = {k: np.asarray(v) for k, v in reference.setup_inputs().items()}
    expected = np.asarray(reference.reference(**inputs))
    actual = kernel(**inputs)
    err = np.abs(actual - expected).max() / (np.abs(expected).max() + 1e-30)
    print("Relative error:", err)



# revision 10
# speedup vs baseline: 1.7703x; 1.7703x over previous
"""Trainium2 Bass kernel for a dense transformer block (B=2, T=2048, C=1024, H=16).

v2 over the v1 baseline:
  - bf16 activations/weights everywhere on the moving path (PSUM stays f32;
    LN stat rows stay f32/f32r -- f32r moving >=256 wide is also full rate).
  - QKV + attn-proj weights SBUF-resident (loaded once per program);
    fc/mlp-proj weights streamed per body, double-buffered.
  - Per-batch AllToAll (two 0.5MB bf16 collectives); the first is hidden
    under batch-1 attention.
  - Paired QK->exp batching: one [128,1024] Exp per two k-blocks; causal
    band masks are pair-sized [128,1024] bf16 DVE multiplies.
  - Bias adds fused into K=2 broadcast matmuls (colsum x nmr + bias x ones),
    PE emission ordered so stats matmuls overlap the LN fixup chain.

Sharding: 8-way tensor parallel over heads for QKV+attention (each core
owns 2 heads over all 4096 tokens); AllToAll switches to token parallelism:
core i owns tokens b0[256i:256i+256] + b1[256i:256i+256] for attn-proj,
LN2 and the MLP.
"""

import math
import sys
from contextlib import ExitStack

import numpy as np

for _p in ("/opt/trn_rl_repo",):
    if _p not in sys.path:
        sys.path.insert(0, _p)

import concourse.bacc as bacc
import concourse.mybir as mybir
import concourse.tile as tile
from concourse.bass_utils import run_bass_kernel_spmd

F32 = mybir.dt.float32
F32R = mybir.dt.float32r
BF16 = mybir.dt.bfloat16
NPBF16 = mybir.dt.np(mybir.dt.bfloat16)

B, T, C = 2, 2048, 1024
H, HD = 16, 64
TT = B * T              # 4096 flat tokens (b0: 0..2047, b1: 2048..4095)
NCORES = 8
CHUNK = 512             # tokens per core for the MLP part (256 per batch)
NC_BLK = C // 128       # 8 feature blocks
NF_BLK = 4 * C // 128   # 32 mlp-hidden blocks
EPS = 1e-5


def build_program(repeat=1, phases=99):
    nc = bacc.Bacc("TRN2", target_bir_lowering=False, debug=False,
                   num_devices=NCORES)

    # ---- I/O (big tensors bf16, per-partition-contiguous layouts) ----
    xT = nc.dram_tensor("xT", [128, NC_BLK, TT], BF16, kind="ExternalInput")
    xc_in = nc.dram_tensor("xc_in", [128, NC_BLK, CHUNK], BF16,
                           kind="ExternalInput")
    wqkv = nc.dram_tensor("wqkv", [128, NC_BLK, 3, 128], BF16,
                          kind="ExternalInput")
    cbqkv = nc.dram_tensor("cbqkv", [1, 6, 128], BF16, kind="ExternalInput")
    wproj = nc.dram_tensor("wproj", [128, NC_BLK, NC_BLK, 128], BF16,
                           kind="ExternalInput")
    bproj = nc.dram_tensor("bproj", [128, NC_BLK], F32, kind="ExternalInput")
    wfc = nc.dram_tensor("wfc", [128, NF_BLK, NC_BLK, 128], BF16,
                         kind="ExternalInput")
    bfc = nc.dram_tensor("bfc", [128, NF_BLK], F32, kind="ExternalInput")
    wmlp = nc.dram_tensor("wmlp", [NC_BLK, 128, NF_BLK, 128], BF16,
                          kind="ExternalInput")
    bmlp = nc.dram_tensor("bmlp", [128, NC_BLK], F32, kind="ExternalInput")
    ones_in = nc.dram_tensor("ones_in", [128, 512], BF16,
                             kind="ExternalInput")
    cmask = nc.dram_tensor("cmask", [128, 4, 512], BF16, kind="ExternalInput")
    identin = nc.dram_tensor("identin", [128, 128], BF16,
                             kind="ExternalInput")
    epsin = nc.dram_tensor("epsin", [128, 1], F32, kind="ExternalInput")
    yout = nc.dram_tensor("yout", [128, NC_BLK, CHUNK], F32,
                          kind="ExternalOutput")

    AFT = mybir.ActivationFunctionType
    ALU = mybir.AluOpType

    with tile.TileContext(nc) as tc, ExitStack() as top:
        psum = top.enter_context(tc.tile_pool(name="psum", bufs=1,
                                              space="PSUM"))
        consts = top.enter_context(tc.tile_pool(name="consts", bufs=1))
        wres = top.enter_context(tc.tile_pool(name="wres", bufs=1))
        rows_pool = top.enter_context(tc.tile_pool(name="rows", bufs=4))
        bcast_pool = top.enter_context(tc.tile_pool(name="bcast", bufs=3))
        work = top.enter_context(tc.tile_pool(name="work", bufs=4))
        dram = top.enter_context(tc.tile_pool(name="dram", bufs=1,
                                              space="DRAM"))

        # ---- constants ----
        ident = consts.tile([128, 128], BF16)
        nc.sync.dma_start(out=ident, in_=identin[:, :])
        ones_bf = consts.tile([128, 512], BF16)
        nc.sync.dma_start(out=ones_bf, in_=ones_in[:, :])
        ones_col = ones_bf[:, 0:1]
        eps_col = consts.tile([128, 1], F32)
        nc.sync.dma_start(out=eps_col, in_=epsin[:, :])
        masks = consts.tile([128, 2, 1024], BF16)  # two band pair-masks
        nc.sync.dma_start(out=masks,
                          in_=cmask.ap().rearrange("p (a b) t -> p a (b t)",
                                                   a=2))
        sb_cbq = consts.tile([1, 6, 128], BF16)
        nc.sync.dma_start(out=sb_cbq, in_=cbqkv[:, :, :])
        sb_bproj = consts.tile([128, NC_BLK], F32)
        nc.sync.dma_start(out=sb_bproj, in_=bproj[:, :])
        sb_bfc = consts.tile([128, NF_BLK], F32)
        nc.sync.dma_start(out=sb_bfc, in_=bfc[:, :])
        sb_bmlp = consts.tile([128, NC_BLK], F32)
        nc.sync.dma_start(out=sb_bmlp, in_=bmlp[:, :])

        # ---- resident weights (loaded once per program) ----
        wq_all = wres.tile([128, NC_BLK, 3, 128], BF16, name="wq_all")
        nc.sync.dma_start(out=wq_all, in_=wqkv[:, :, :, :])
        wp_all = wres.tile([128, NC_BLK, NC_BLK, 128], BF16, name="wp_all")
        nc.sync.dma_start(out=wp_all, in_=wproj[:, :, :, :])

        def bcast_row(row_ap, nparts, dtype, tag="bc"):
            """Broadcast a [1, 512] SBUF row to [nparts, 512] via a K=1 PE
            outer product with a ones row, evacuated to SBUF by DVE."""
            ps = psum.tile([128, 512], F32, tag="mm", bufs=2, name="ps_bc")
            nc.tensor.matmul(ps[0:nparts, :], ones_bf[0:1, 0:nparts], row_ap,
                             start=True, stop=True)
            out = bcast_pool.tile([nparts, 512], dtype, tag=tag,
                                  name="bc_row")
            with nc.allow_low_precision(reason="broadcast copy"):
                nc.vector.tensor_copy(out, ps[0:nparts, :])
            return out

        def ln_stats_rows(srcs, sqs):
            """srcs/sqs: NC_BLK [128, 512] bf16 APs (feature blocks of one
            512-token chunk and their elementwise squares). Returns
            (rstd_row [1,512] f32r, nm1 [2,512] bf16 = (-mu*rstd; ones))."""
            ps_s = psum.tile([65, 512], F32, tag="av", bufs=2, name="ps_s")
            ps_q = psum.tile([65, 512], F32, tag="av", bufs=2, name="ps_q")
            for c in range(NC_BLK):
                nc.tensor.matmul(ps_s[0:1, :], ones_col, srcs[c],
                                 start=(c == 0), stop=(c == NC_BLK - 1))
            for c in range(NC_BLK):
                nc.tensor.matmul(ps_q[0:1, :], ones_col, sqs[c],
                                 start=(c == 0), stop=(c == NC_BLK - 1))
            mu = rows_pool.tile([1, 512], F32, tag="r")
            nc.vector.tensor_scalar_mul(mu, ps_s[0:1, :], 1.0 / C)
            ex2 = rows_pool.tile([1, 512], F32, tag="r")
            nc.vector.tensor_scalar_mul(ex2, ps_q[0:1, :], 1.0 / C)
            var = rows_pool.tile([1, 512], F32, tag="r")
            musq = rows_pool.tile([1, 512], F32, tag="r")
            nc.vector.tensor_mul(musq, mu, mu)
            nc.vector.tensor_sub(var, ex2, musq)
            sd = rows_pool.tile([1, 512], F32, tag="r")
            nc.scalar.activation(out=sd, in_=var, func=AFT.Sqrt,
                                 bias=eps_col[0:1, 0:1])
            rstd = rows_pool.tile([1, 512], BF16, tag="r")
            with nc.allow_low_precision(reason="bf16 rstd row"):
                nc.vector.reciprocal(rstd, sd)
            nmr = rows_pool.tile([1, 512], BF16, tag="nm", bufs=2)
            with nc.allow_low_precision(reason="bf16 bcast rows"):
                nc.vector.tensor_mul(nmr, mu, rstd)
                nc.vector.tensor_scalar_mul(nmr, nmr, -1.0)
            return rstd, nmr

        def emit_body(rep):
            with ExitStack() as body_scope:
                body = body_scope.enter_context(
                    tc.tile_pool(name=f"body{rep}", bufs=1))
                xct = body.tile([128, NC_BLK, CHUNK], BF16, name="xct")
                residT = body.tile([128, NC_BLK, CHUNK], BF16, name="residT")
                ln2T = body.tile([128, NC_BLK, CHUNK], BF16, name="ln2T")
                hT = body.tile([128, NF_BLK, CHUNK], BF16, name="hT")

                # residual-chunk prefetch: per-core input, independent of
                # anything in-body
                nc.sync.dma_start(out=xct, in_=xc_in[:, :, :])

                with ExitStack() as attn_scope:
                    attn_pool = attn_scope.enter_context(
                        tc.tile_pool(name=f"attn{rep}", bufs=1))
                    # Q,K transposed; rows 0:64 head A, 64:128 head B
                    qkT = attn_pool.tile([128, 2, TT], BF16, name="qkT")
                    # V in natural [token, dim] blocks + appended ones col
                    vones = attn_pool.tile([128, 2, TT // 128, 65], BF16,
                                           name="vones")
                    yT = attn_pool.tile([128, B, T], BF16, name="yT")
                    nc.vector.tensor_copy(
                        vones[:, :, :, 64:65].rearrange(
                            "p a b k -> p (a b k)"),
                        ones_bf[:, 0:64])

                    # ========== Phase 1: LN1 stats + QKV ==========
                    with ExitStack() as p1_scope:
                        xc_pool = p1_scope.enter_context(
                            tc.tile_pool(name=f"xcp{rep}", bufs=2))
                        for qi in range(8):
                            csl = slice(512 * qi, 512 * (qi + 1))
                            xTc = xc_pool.tile([128, NC_BLK, 512], BF16,
                                               tag="xTc", name="xTc")
                            nc.sync.dma_start(out=xTc, in_=xT[:, :, csl])
                            srcs = [xTc[:, c, :] for c in range(NC_BLK)]
                            sq = xc_pool.tile([128, NC_BLK, 512], BF16,
                                              tag="sq", name="sq")
                            with nc.allow_low_precision(reason="bf16 sq"):
                                for c in range(NC_BLK):
                                    nc.vector.tensor_mul(sq[:, c, :],
                                                         srcs[c], srcs[c])
                            # QKV matmuls first (independent of stats) so PE
                            # is busy while DVE produces squares/stats rows
                            ps_j = []
                            for j in range(3):
                                ps = psum.tile([128, 2, 512], F32, tag="qk",
                                               bufs=2, name="ps_qkv")
                                for c in range(NC_BLK):
                                    nc.tensor.matmul(ps[:, 0, :],
                                                     wq_all[:, c, j, :],
                                                     srcs[c],
                                                     start=(c == 0),
                                                     stop=(c == NC_BLK - 1))
                                ps_j.append(ps)
                            rstd, nmr = ln_stats_rows(
                                srcs, [sq[:, c, :] for c in range(NC_BLK)])
                            rstd_b = bcast_row(rstd[0:1, :], 128, BF16)
                            for j in range(3):
                                # ps2 = colsum_j x nmr + bias_j x ones
                                nc.tensor.matmul(ps_j[j][:, 1, :],
                                                 sb_cbq[:, j, :], nmr[:, :],
                                                 start=True, stop=False)
                                nc.tensor.matmul(ps_j[j][:, 1, :],
                                                 sb_cbq[:, 3 + j, :],
                                                 ones_bf[0:1, 0:512],
                                                 start=False, stop=True)
                            for j in range(3):
                                t1 = work.tile([128, 512], BF16, tag="wk",
                                               name="t1")
                                with nc.allow_low_precision(reason="bf16"):
                                    nc.vector.tensor_mul(t1, ps_j[j][:, 0, :],
                                                         rstd_b)
                                    if j < 2:
                                        nc.vector.tensor_add(
                                            qkT[:, j, csl], t1,
                                            ps_j[j][:, 1, :])
                                    else:
                                        vch = work.tile([128, 512], BF16,
                                                        tag="vch", bufs=2,
                                                        name="vch")
                                        nc.vector.tensor_add(
                                            vch, t1, ps_j[j][:, 1, :])
                                        for kb in range(4):
                                            ps_t = psum.tile(
                                                [128, 128], BF16,
                                                tag="mm", bufs=2,
                                                name="ps_tr")
                                            nc.tensor.transpose(
                                                ps_t,
                                                vch[:, 128 * kb:
                                                    128 * (kb + 1)],
                                                ident)
                                            gb = 4 * qi + kb
                                            for hh in range(2):
                                                nc.vector.tensor_copy(
                                                    vones[:, hh, gb, 0:64],
                                                    ps_t[:, 64 * hh:
                                                         64 * hh + 64])
                    if phases <= 1:
                        return

                    # ====== Phase 2: causal attention, b-major ======
                    a2a_in = [dram.tile([NCORES, 128, 256], BF16,
                                        name=f"a2a_in{b}") for b in range(B)]
                    a2a_out = [dram.tile([NCORES, 128, 256], BF16,
                                         name=f"a2a_out{b}")
                               for b in range(B)]
                    inv_sqrt_hd = 1.0 / math.sqrt(HD)
                    for b in range(B):
                        for hh in range(2):
                            hsl = slice(64 * hh, 64 * hh + 64)
                            for ql in range(4):
                                npair = 2 * ql + 2
                                q_sl = slice(T * b + 512 * ql,
                                             T * b + 512 * (ql + 1))
                                ps_y = psum.tile([65, 512], F32, tag="av",
                                                 bufs=2, name="ps_y")
                                for pp in range(npair):
                                    pair = psum.tile([128, 2, 512], F32,
                                                     tag="qk", bufs=2,
                                                     name="ps_qk")
                                    for half in range(2):
                                        k = 2 * pp + half
                                        k_sl = slice(T * b + 128 * k,
                                                     T * b + 128 * (k + 1))
                                        nc.tensor.matmul(
                                            pair[:, half, :],
                                            qkT[hsl, 1, k_sl],
                                            qkT[hsl, 0, q_sl],
                                            start=True, stop=True)
                                    est = work.tile([128, 2, 512], BF16,
                                                    tag="est", bufs=3,
                                                    name="est")
                                    nc.scalar.activation(
                                        out=est.rearrange("p a t -> p (a t)"),
                                        in_=pair.rearrange("p a t -> p (a t)"),
                                        func=AFT.Exp, scale=inv_sqrt_hd)
                                    m2 = pp - 2 * ql  # 0/1 for band pairs
                                    if m2 >= 0:
                                        with nc.allow_low_precision(
                                                reason="bf16 mask"):
                                            nc.vector.tensor_mul(
                                                est.rearrange(
                                                    "p a t -> p (a t)"),
                                                est.rearrange(
                                                    "p a t -> p (a t)"),
                                                masks[:, m2, :])
                                    for half in range(2):
                                        k = 2 * pp + half
                                        nc.tensor.matmul(
                                            ps_y[0:65, :],
                                            vones[:, hh,
                                                  (T * b) // 128 + k, :],
                                            est[:, half, :],
                                            start=(pp == 0 and half == 0),
                                            stop=(pp == npair - 1
                                                  and half == 1))
                                # normalize via recip of sum row (part 64)
                                srow = rows_pool.tile([1, 512], BF16,
                                                      tag="sr", bufs=2,
                                                      name="srow")
                                with nc.allow_low_precision(reason="bf16"):
                                    nc.vector.reciprocal(srow[0:1, :],
                                                         ps_y[64:65, :])
                                rb = bcast_row(srow[0:1, :], 64, F32,
                                               tag="rb")
                                with nc.allow_low_precision(reason="bf16 y"):
                                    nc.vector.tensor_mul(
                                        yT[64 * hh:64 * hh + 64, b,
                                           512 * ql:512 * (ql + 1)],
                                        ps_y[0:64, :], rb[0:64, :])
                        if phases <= 3:
                            continue
                        # ======== per-batch AllToAll ========
                        nc.sync.dma_start(
                            out=a2a_in[b].rearrange("j p t -> p j t"),
                            in_=yT[:, b, :].rearrange("p (j t) -> p j t",
                                                      j=NCORES))
                        nc.gpsimd.collective_compute(
                            "AllToAll", ALU.bypass,
                            replica_groups=[list(range(NCORES))],
                            ins=[a2a_in[b][:]], outs=[a2a_out[b][:]],
                        )
                    if phases <= 4:
                        return

                    # ===== Phase 4: attn proj + residual =====
                    # y-full load on the ACT HWDGE queue (collective-
                    # dependent; keeps the SP queue clear)
                    yfull = attn_pool.tile([128, NC_BLK, CHUNK], BF16,
                                           name="yfull")
                    for b in range(B):
                        nc.scalar.dma_start(
                            out=yfull[:, :, 256 * b:256 * (b + 1)],
                            in_=a2a_out[b].rearrange("j p t -> p j t"))
                    for co in range(NC_BLK):
                        ps = psum.tile([128, 2, 512], F32, tag="qk", bufs=2,
                                       name="ps_pj")
                        for ci in range(NC_BLK):
                            nc.tensor.matmul(ps[:, 0, :],
                                             wp_all[:, ci, co, :],
                                             yfull[:, ci, :],
                                             start=(ci == 0),
                                             stop=(ci == NC_BLK - 1))
                        with nc.allow_low_precision(reason="bf16 resid"):
                            nc.vector.scalar_tensor_tensor(
                                out=residT[:, co, :], in0=ps[:, 0, :],
                                scalar=sb_bproj[:, co:co + 1],
                                in1=xct[:, co, :], op0=ALU.add, op1=ALU.add)
                if phases <= 5:
                    return

                # ===== Phase 5: LN2 =====
                sq2 = work.tile([128, NC_BLK, 512], BF16, tag="sq2", bufs=1,
                                name="sq2")
                with nc.allow_low_precision(reason="bf16 sq"):
                    for c in range(NC_BLK):
                        nc.vector.tensor_mul(sq2[:, c, :], residT[:, c, :],
                                             residT[:, c, :])
                rstd2, nmr2 = ln_stats_rows(
                    [residT[:, c, :] for c in range(NC_BLK)],
                    [sq2[:, c, :] for c in range(NC_BLK)])
                rstd2_b = bcast_row(rstd2[0:1, :], 128, BF16)
                nmr2_b = bcast_row(nmr2[0:1, :], 128, BF16, tag="bc2")
                with nc.allow_low_precision(reason="bf16 ln2"):
                    for c in range(NC_BLK):
                        t = work.tile([128, 512], BF16, tag="wk", name="t5")
                        nc.vector.tensor_mul(t, residT[:, c, :], rstd2_b)
                        nc.vector.tensor_add(ln2T[:, c, :], t, nmr2_b)
                if phases <= 6:
                    return

                # ===== Phase 6/7: MLP (fc/gelu then proj), streamed ======
                with ExitStack() as p6_scope:
                    p6_pool = p6_scope.enter_context(
                        tc.tile_pool(name=f"p6_{rep}", bufs=1))
                    for f in range(NF_BLK):
                        wf_t = p6_pool.tile([128, NC_BLK, 128], BF16,
                                            tag="wf", bufs=3, name="wf")
                        nc.sync.dma_start(out=wf_t, in_=wfc[:, f, :, :])
                        ps = psum.tile([128, 2, 512], F32, tag="qk", bufs=2,
                                       name="ps_fc")
                        for c in range(NC_BLK):
                            nc.tensor.matmul(ps[:, 0, :], wf_t[:, c, :],
                                             ln2T[:, c, :],
                                             start=(c == 0),
                                             stop=(c == NC_BLK - 1))
                        nc.scalar.activation(out=hT[:, f, :], in_=ps[:, 0, :],
                                             func=AFT.Gelu,
                                             bias=sb_bfc[:, f:f + 1])
                    if phases <= 7:
                        return

                    for co in range(NC_BLK):
                        wm = p6_pool.tile([128, NF_BLK, 128], BF16,
                                          tag="wm", bufs=2, name="wm")
                        nc.sync.dma_start(out=wm, in_=wmlp[co, :, :, :])
                        ps = psum.tile([128, 2, 512], F32, tag="qk", bufs=2,
                                       name="ps_mp")
                        for f in range(NF_BLK):
                            nc.tensor.matmul(ps[:, 0, :], wm[:, f, :],
                                             hT[:, f, :],
                                             start=(f == 0),
                                             stop=(f == NF_BLK - 1))
                        yo = work.tile([128, 512], F32, tag="yo", bufs=2,
                                       name="yo")
                        nc.vector.scalar_tensor_tensor(
                            out=yo, in0=ps[:, 0, :],
                            scalar=sb_bmlp[:, co:co + 1],
                            in1=residT[:, co, :], op0=ALU.add, op1=ALU.add)
                        nc.sync.dma_start(out=yout[:, co, :], in_=yo)

        for _rep in range(repeat):
            emit_body(_rep)

    nc.compile()
    return nc


_NC_CACHE = {}


def _get_program(repeat=1):
    if repeat not in _NC_CACHE:
        _NC_CACHE[repeat] = build_program(repeat)
    return _NC_CACHE[repeat]


def prepare_inputs(x, ln1_g, ln1_b, w_attn, b_attn, w_attn_proj, b_attn_proj,
                   ln2_g, ln2_b, w_fc, b_fc, w_mlp_proj, b_mlp_proj):
    """Host-side fold/slice/block. Returns in_maps (list of 8 dicts)."""
    f = np.float32
    bf = NPBF16
    x = np.asarray(x, f)
    # fold LN1 gain into w_attn, LN2 gain into w_fc (exact: reference
    # applies g/b after normalization; W'.T @ (g*xn + b) = (g*W)'.T @ xn
    # + (b @ W))
    w_attn_e = (np.asarray(ln1_g, f)[:, None] * np.asarray(w_attn, f))
    b_attn_e = np.asarray(ln1_b, f) @ np.asarray(w_attn, f) + \
        np.asarray(b_attn, f)
    w_fc_e = (np.asarray(ln2_g, f)[:, None] * np.asarray(w_fc, f))
    b_fc_e = np.asarray(ln2_b, f) @ np.asarray(w_fc, f) + np.asarray(b_fc, f)
    colsum = w_attn_e.sum(axis=0, dtype=np.float64).astype(f)

    xT = np.concatenate([x[0].T, x[1].T], axis=1)          # [C, 4096]
    xT_blk = np.ascontiguousarray(
        xT.reshape(NC_BLK, 128, TT).transpose(1, 0, 2)).astype(bf)

    # wproj: [C, C] -> [p(ci-row), ci, co, k]
    wp = np.ascontiguousarray(
        np.asarray(w_attn_proj, f).reshape(NC_BLK, 128, NC_BLK, 128)
        .transpose(1, 0, 2, 3)).astype(bf)
    # wfc: [C, 4C] -> [p(c-row), f, c, k]  (per-f slices contiguous)
    wf = np.ascontiguousarray(
        w_fc_e.reshape(NC_BLK, 128, NF_BLK, 128)
        .transpose(1, 2, 0, 3)).astype(bf)
    # wmlp: [4C, C] -> [co, p(f-row), f, k]
    wm = np.ascontiguousarray(
        np.asarray(w_mlp_proj, f).reshape(NF_BLK, 128, NC_BLK, 128)
        .transpose(2, 1, 0, 3)).astype(bf)

    def rows_t(v, nb):
        return np.ascontiguousarray(np.asarray(v, f).reshape(nb, 128).T)

    bproj_t = rows_t(b_attn_proj, NC_BLK)
    bfc_t = rows_t(b_fc_e, NF_BLK)
    bmlp_t = rows_t(b_mlp_proj, NC_BLK)

    ones_arr = np.ones((128, 512), bf)
    ident_arr = np.eye(128).astype(bf)
    eps_arr = np.full((128, 1), EPS, f)
    # causal mask m: [p, c] valid (1.0) iff p + 128*m <= c
    p_idx = np.arange(128)[:, None]
    c_idx = np.arange(512)[None, :]
    cmask_arr = np.ascontiguousarray(np.stack(
        [(p_idx + 128 * m <= c_idx).astype(f) for m in range(4)],
        axis=1)).astype(bf)

    in_maps = []
    for i in range(NCORES):
        qcols = slice(128 * i, 128 * (i + 1))
        kcols = slice(C + 128 * i, C + 128 * (i + 1))
        vcols = slice(2 * C + 128 * i, 2 * C + 128 * (i + 1))
        wq = np.empty((128, NC_BLK, 3, 128), f)
        for c in range(NC_BLK):
            rsl = slice(128 * c, 128 * (c + 1))
            wq[:, c, 0, :] = w_attn_e[rsl, qcols]
            wq[:, c, 1, :] = w_attn_e[rsl, kcols]
            wq[:, c, 2, :] = w_attn_e[rsl, vcols]
        cb = np.empty((1, 6, 128), f)
        for j, sl in enumerate((qcols, kcols, vcols)):
            cb[0, j, :] = colsum[sl]
            cb[0, 3 + j, :] = b_attn_e[sl]
        # per-core residual chunk: b0[256i:256(i+1)] ++ b1[256i:256(i+1)]
        xc = np.concatenate(
            [xT_blk[:, :, 256 * i:256 * (i + 1)],
             xT_blk[:, :, 2048 + 256 * i:2048 + 256 * (i + 1)]], axis=2)
        in_maps.append({
            "ones_in": ones_arr,
            "cmask": cmask_arr,
            "identin": ident_arr,
            "epsin": eps_arr,
            "xT": xT_blk,
            "xc_in": np.ascontiguousarray(xc),
            "wqkv": wq.astype(bf),
            "cbqkv": cb.astype(bf),
            "wproj": wp,
            "bproj": bproj_t,
            "wfc": wf,
            "bfc": bfc_t,
            "wmlp": wm,
            "bmlp": bmlp_t,
        })
    return in_maps


def assemble_output(results):
    out = np.empty((B, T, C), np.float32)
    for i in range(NCORES):
        yo = results[i]["yout"]                      # [128, 8, 512]
        y = yo.transpose(1, 0, 2).reshape(C, CHUNK)  # [feature, 512]
        out[0, 256 * i:256 * (i + 1), :] = y[:, 0:256].T
        out[1, 256 * i:256 * (i + 1), :] = y[:, 256:512].T
    return out


def kernel(**inputs):
    nc = _get_program()
    in_maps = prepare_inputs(**inputs)
    res = run_bass_kernel_spmd(nc, in_maps, list(range(NCORES)))
    return assemble_output(res.results)


if __name__ == "__main__":
    import reference
    inputs = {k: np.asarray(v) for k, v in reference.setup_inputs().items()}
    expected = np.asarray(reference.reference(**inputs))
    actual = kernel(**inputs)
    err = np.abs(actual - expected).max() / (np.abs(expected).max() + 1e-30)
    print("Relative error:", err)
